# revision 2
# baseline (speedup 1.0000x reference)
"""ApplyCoeffs (bilateral-grid style per-pixel affine) on 8 TRN2 NeuronCores.

out[n,o,h,w] = sum_i x_aug[n,i,h,w] * coeff[n, i*31+o, h, w],
x_aug = [R, G, B, 1].  Purely pointwise per pixel -> data-parallel shard
over (N, H/2) across 8 cores, no communication.

Per-core shard: coeff [4,31,131072] f32 (65MB), x [3,131072], out [31,131072].
SBUF layout: 128 partitions x 1024 pixels.  Output channels processed in
groups of G=4: one 8MB DMA loads the 4*G coeff planes, 6 DVE tensor_tensor
ops compute og = c0*R + c1*G + c2*B + c3 for the group, one 2MB DMA stores.
"""

import sys

for _p in ("/opt/trn_rl_repo",):
    if _p not in sys.path:
        sys.path.insert(0, _p)

import numpy as np

N, H, W = 4, 512, 512
CI, CO = 4, 31
NCORES = 8
HS = H // 2            # rows per core
P = HS * W             # pixels per core shard
PPART = P // 128       # pixels per SBUF partition
GROUPS = [4] * 7 + [3]
GMAX = 4

_nc_cache = None


def _build():
    from concourse import bacc, mybir, tile

    nc = bacc.Bacc("TRN2", target_bir_lowering=False, debug=False,
                   num_devices=NCORES)
    coeff = nc.dram_tensor("coeff", [CI, CO, P], mybir.dt.float32,
                           kind="ExternalInput")
    x = nc.dram_tensor("x", [3, P], mybir.dt.float32, kind="ExternalInput")
    out = nc.dram_tensor("out", [CO, P], mybir.dt.float32,
                         kind="ExternalOutput")

    with tile.TileContext(nc) as tc:
        with tc.tile_pool(name="cpool", bufs=2) as cpool, \
             tc.tile_pool(name="opool", bufs=2) as opool, \
             tc.tile_pool(name="spool", bufs=1) as spool, \
             tc.tile_pool(name="xpool", bufs=1) as xpool:
            xt = xpool.tile([128, 3, PPART], mybir.dt.float32)
            nc.sync.dma_start(
                out=xt, in_=x.ap().rearrange("c (p j) -> p c j", p=128))

            o0 = 0
            for G in GROUPS:
                ct = cpool.tile([128, CI, GMAX, PPART], mybir.dt.float32,
                                tag="c", name=f"c{o0}")
                for i in range(CI):
                    nc.sync.dma_start(
                        out=ct[:, i, :G, :],
                        in_=coeff.ap()[i, o0:o0 + G, :].rearrange(
                            "g (p j) -> p g j", p=128))

                og = opool.tile([128, GMAX, PPART], mybir.dt.float32,
                                tag="og", name=f"og{o0}")
                t = spool.tile([128, GMAX, PPART], mybir.dt.float32,
                               tag="t", name=f"t{o0}")
                ogv = og[:, :G, :]
                tv = t[:, :G, :]
                Rb = xt[:, 0:1, :].broadcast_to([128, G, PPART])
                Gb = xt[:, 1:2, :].broadcast_to([128, G, PPART])
                Bb = xt[:, 2:3, :].broadcast_to([128, G, PPART])

                nc.vector.tensor_mul(out=ogv, in0=ct[:, 0, :G, :], in1=Rb)
                nc.vector.tensor_mul(out=tv, in0=ct[:, 1, :G, :], in1=Gb)
                nc.vector.tensor_add(out=ogv, in0=ogv, in1=tv)
                nc.vector.tensor_mul(out=tv, in0=ct[:, 2, :G, :], in1=Bb)
                nc.vector.tensor_add(out=ogv, in0=ogv, in1=tv)
                nc.vector.tensor_add(out=ogv, in0=ogv, in1=ct[:, 3, :G, :])

                nc.sync.dma_start(
                    out=out.ap()[o0:o0 + G, :].rearrange(
                        "g (p j) -> p g j", p=128),
                    in_=ogv)
                o0 += G

    nc.compile()
    return nc


def _get_nc():
    global _nc_cache
    if _nc_cache is None:
        _nc_cache = _build()
    return _nc_cache


def _make_in_maps(coeff, full_res_input):
    coeff = np.asarray(coeff, dtype=np.float32)
    x = np.asarray(full_res_input, dtype=np.float32)
    in_maps = []
    for k in range(NCORES):
        n, h0 = k // 2, (k % 2) * HS
        cs = np.ascontiguousarray(coeff[n, :, h0:h0 + HS, :]).reshape(
            CI, CO, P)
        xs = np.ascontiguousarray(x[n, :, h0:h0 + HS, :]).reshape(3, P)
        in_maps.append({"coeff": cs, "x": xs})
    return in_maps


def _gather(results):
    out = np.empty((N, CO, H, W), np.float32)
    for k in range(NCORES):
        n, h0 = k // 2, (k % 2) * HS
        out[n, :, h0:h0 + HS, :] = results[k]["out"].reshape(CO, HS, W)
    return out


def _run(in_maps, trace=False):
    from concourse import bass_utils
    return bass_utils.run_bass_kernel_spmd(
        _get_nc(), in_maps, core_ids=list(range(NCORES)), trace=trace)


def kernel(coeff, full_res_input):
    res = _run(_make_in_maps(coeff, full_res_input))
    return _gather(res.results)


# revision 4
# speedup vs baseline: 1.0098x; 1.0098x over previous
"""ApplyCoeffs (bilateral-grid style per-pixel affine) on 8 TRN2 NeuronCores.

out[n,o,h,w] = sum_i x_aug[n,i,h,w] * coeff[n, i*31+o, h, w],
x_aug = [R, G, B, 1].  Purely pointwise per pixel -> data-parallel shard
over (N, H/2) across 8 cores, no communication.

Per-core shard: coeff [4,31,131072] f32 (65MB), x [3,131072], out [31,131072].
SBUF layout: 128 partitions x 1024 pixels.  Output channels processed in
groups of G=4: one 8MB DMA loads the 4*G coeff planes, 6 DVE tensor_tensor
ops compute og = c0*R + c1*G + c2*B + c3 for the group, one 2MB DMA stores.
"""

import sys

for _p in ("/opt/trn_rl_repo",):
    if _p not in sys.path:
        sys.path.insert(0, _p)

import numpy as np

N, H, W = 4, 512, 512
CI, CO = 4, 31
NCORES = 8
HS = H // 2            # rows per core
P = HS * W             # pixels per core shard
PPART = P // 128       # pixels per SBUF partition
GROUPS = [4] * 7 + [2, 1]
GMAX = 4

_nc_cache = None


def _build():
    from concourse import bacc, mybir, tile

    nc = bacc.Bacc("TRN2", target_bir_lowering=False, debug=False,
                   num_devices=NCORES)
    coeff = nc.dram_tensor("coeff", [CI, CO, P], mybir.dt.float32,
                           kind="ExternalInput")
    x = nc.dram_tensor("x", [3, P], mybir.dt.float32, kind="ExternalInput")
    out = nc.dram_tensor("out", [CO, P], mybir.dt.float32,
                         kind="ExternalOutput")

    with tile.TileContext(nc) as tc:
        with tc.tile_pool(name="cpool", bufs=2) as cpool, \
             tc.tile_pool(name="opool", bufs=2) as opool, \
             tc.tile_pool(name="spool", bufs=1) as spool, \
             tc.tile_pool(name="xpool", bufs=1) as xpool:
            xt = xpool.tile([128, 3, PPART], mybir.dt.float32)
            nc.sync.dma_start(
                out=xt, in_=x.ap().rearrange("c (p j) -> p c j", p=128))

            o0 = 0
            for G in GROUPS:
                # One coeff tile per input channel: per-i tags mean the
                # slot for group g+1's channel-i load frees as soon as
                # group g-1's *reader of channel i* is done, keeping the
                # load pipeline deep without extra SBUF.
                cts = []
                for i in range(CI):
                    ci = cpool.tile([128, GMAX, PPART], mybir.dt.float32,
                                    tag=f"c{i}", name=f"c{i}_{o0}")
                    nc.sync.dma_start(
                        out=ci[:, :G, :],
                        in_=coeff.ap()[i, o0:o0 + G, :].rearrange(
                            "g (p j) -> p g j", p=128))
                    cts.append(ci)

                og = opool.tile([128, GMAX, PPART], mybir.dt.float32,
                                tag="og", name=f"og{o0}")
                t = spool.tile([128, GMAX, PPART], mybir.dt.float32,
                               tag="t", name=f"t{o0}")
                ogv = og[:, :G, :]
                tv = t[:, :G, :]
                Rb = xt[:, 0:1, :].broadcast_to([128, G, PPART])
                Gb = xt[:, 1:2, :].broadcast_to([128, G, PPART])
                Bb = xt[:, 2:3, :].broadcast_to([128, G, PPART])

                nc.vector.tensor_mul(out=ogv, in0=cts[0][:, :G, :], in1=Rb)
                nc.vector.tensor_mul(out=tv, in0=cts[1][:, :G, :], in1=Gb)
                nc.vector.tensor_add(out=ogv, in0=ogv, in1=tv)
                nc.vector.tensor_mul(out=tv, in0=cts[2][:, :G, :], in1=Bb)
                nc.vector.tensor_add(out=ogv, in0=ogv, in1=tv)
                nc.vector.tensor_add(out=ogv, in0=ogv, in1=cts[3][:, :G, :])

                # Store on the ACT HWDGE ring so a store waiting on DVE
                # never head-of-line-blocks the next group's loads on SP.
                nc.scalar.dma_start(
                    out=out.ap()[o0:o0 + G, :].rearrange(
                        "g (p j) -> p g j", p=128),
                    in_=ogv)
                o0 += G

    nc.compile()
    return nc


def _get_nc():
    global _nc_cache
    if _nc_cache is None:
        _nc_cache = _build()
    return _nc_cache


def _make_in_maps(coeff, full_res_input):
    coeff = np.asarray(coeff, dtype=np.float32)
    x = np.asarray(full_res_input, dtype=np.float32)
    in_maps = []
    for k in range(NCORES):
        n, h0 = k // 2, (k % 2) * HS
        cs = np.ascontiguousarray(coeff[n, :, h0:h0 + HS, :]).reshape(
            CI, CO, P)
        xs = np.ascontiguousarray(x[n, :, h0:h0 + HS, :]).reshape(3, P)
        in_maps.append({"coeff": cs, "x": xs})
    return in_maps


def _gather(results):
    out = np.empty((N, CO, H, W), np.float32)
    for k in range(NCORES):
        n, h0 = k // 2, (k % 2) * HS
        out[n, :, h0:h0 + HS, :] = results[k]["out"].reshape(CO, HS, W)
    return out


def _run(in_maps, trace=False):
    from concourse import bass_utils
    return bass_utils.run_bass_kernel_spmd(
        _get_nc(), in_maps, core_ids=list(range(NCORES)), trace=trace)


def kernel(coeff, full_res_input):
    res = _run(_make_in_maps(coeff, full_res_input))
    return _gather(res.results)
